# revision 11
# baseline (speedup 1.0000x reference)
"""Trainium2 Bass kernel for AssociativeMemoryStep (decayed linear attention).

Math (per batch b):
    q_w = softmax(basis @ q_coeffs.T, axis=0)          # (V, C)
    k_w, v_w likewise; o_w = basis @ o_coeffs.T        # (V, C)
    q/k/v = x @ {q,k,v}_w                              # (T, C)
    M_t = d*M_{t-1} + k_t v_t^T ;  r_t = q_t^T M_{t-1} # read-before-write
    out = (r @ o_w.T) * out_scale                      # (T, V)

Strategy: data-parallel over B (8 batches -> 8 NeuronCores). Inside each
core the T=2048 sequence is processed in 16 chunks of L=128 with the
standard chunked linear-attention decomposition:
    r_chunk = (Q*d^j) @ S  +  ((K^T (Q*d^j)) .* DM) applied to V
    S_new   = d^L * S + (K*d^(L-1-s))^T V
where DM[s,j] = d^(-s-1) * [s<j] folds the decay+causal mask into one
elementwise multiply of the (s,j) score matrix.

Layout: x is passed host-transposed as xT (V, T) f32 so the V-contraction
of the projections needs no on-device transpose; a SWDGE casting DMA
loads each 512-column block straight into bf16 SBUF in (p, vb, t) form.
Projections are computed column-major (C on partitions, T free, N=512);
row-major K/V (for the contraction over s) come from xbar transposes of
the projected blocks, alternating between the two HWDGE queues. All
matmuls run in bf16 with fp32 PSUM accumulation.
"""

import numpy as np
import ml_dtypes

B, T, V, C, NB2 = 8, 2048, 1024, 128, 64
L = 128               # recurrence chunk
BLK = 512             # projection block (free dim of proj matmuls)
NBLK = T // BLK       # 4
NCH = BLK // L        # 4 chunks per block

_cache = {}


def _build():
    import concourse.bass as bass
    import concourse.tile as tile
    from concourse import bacc, mybir

    f32 = mybir.dt.float32
    bf16 = mybir.dt.bfloat16
    AF = mybir.ActivationFunctionType
    ALU = mybir.AluOpType

    nc = bacc.Bacc()

    xT_d = nc.declare_dram_parameter("xT", [V, T], f32, isOutput=False)
    basisT_d = nc.declare_dram_parameter("basisT", [NB2, V], bf16, isOutput=False)
    coeff_d = {
        w: nc.declare_dram_parameter(f"{w}cT", [NB2, C], bf16, isOutput=False)
        for w in ("q", "k", "v", "o")
    }
    dq_d = nc.declare_dram_parameter("dq", [128, BLK], f32, isOutput=False)
    dm_d = nc.declare_dram_parameter("dm", [128, 128], f32, isOutput=False)
    dlk_d = nc.declare_dram_parameter("dlk", [128, 1], f32, isOutput=False)
    dl_d = nc.declare_dram_parameter("dl", [128, 1], f32, isOutput=False)
    out_d = nc.declare_dram_parameter("out", [T, V], f32, isOutput=True)

    xT_t = xT_d[:].rearrange("(vb p) t -> p vb t", p=128)  # (128, 8, T)

    with tile.TileContext(nc) as tc:
        with (
            tc.tile_pool(name="singles", bufs=1) as singles,
            tc.tile_pool(name="wtmp", bufs=3) as wtmp,
            tc.tile_pool(name="small", bufs=4) as small,
            tc.tile_pool(name="xt", bufs=3) as xt_pool,
            tc.tile_pool(name="blk", bufs=2) as blk_pool,
            tc.tile_pool(name="rm", bufs=4) as rm_pool,
            tc.tile_pool(name="ch", bufs=3) as ch_pool,
            tc.tile_pool(name="outsb", bufs=3) as out_pool,
            tc.tile_pool(name="pwlog", bufs=1, space="PSUM") as pwlog,
            tc.tile_pool(name="pproj", bufs=2, space="PSUM") as pproj,
            tc.tile_pool(name="pa", bufs=1, space="PSUM") as pa_pool,
            tc.tile_pool(name="pr", bufs=1, space="PSUM") as pr_pool,
            tc.tile_pool(name="ps", bufs=1, space="PSUM") as ps_pool,
        ):
            # ---- small params -> SBUF
            basisT = singles.tile([NB2, V], bf16, tag="basisT")
            nc.gpsimd.dma_start(basisT, basisT_d[:])
            coeff = {}
            for w in ("q", "k", "v", "o"):
                coeff[w] = singles.tile([NB2, C], bf16, tag=f"{w}cT", name=f"{w}cT_sb")
                nc.gpsimd.dma_start(coeff[w], coeff_d[w][:])
            dq = singles.tile([128, BLK], f32, tag="dq")
            nc.gpsimd.dma_start(dq, dq_d[:])
            dm = singles.tile([128, 128], f32, tag="dm")
            nc.gpsimd.dma_start(dm, dm_d[:])
            dlk = singles.tile([128, 1], f32, tag="dlk")
            nc.gpsimd.dma_start(dlk, dlk_d[:])
            dl = singles.tile([128, 1], f32, tag="dl")
            nc.gpsimd.dma_start(dl, dl_d[:])

            # ---- Fourier-parameterized projection weights, on device.
            # logits^T (C,V) = coeffs @ basis.T ; softmax along free (V).
            w_nat = {}   # (128, 8, 128) bf16: w_nat[p, i, c] = w[(128i+p), c]
            o_wT = None  # (C, V) bf16
            for w in ("q", "k", "v", "o"):
                logits = pwlog.tile([128, V], f32, tag="wlog")
                for h in range(V // 512):
                    nc.tensor.matmul(
                        logits[:, h * 512:(h + 1) * 512],
                        coeff[w],
                        basisT[:, h * 512:(h + 1) * 512],
                        start=True, stop=True,
                    )
                if w == "o":
                    o_wT = singles.tile([C, V], bf16, tag="o_wT")
                    nc.scalar.copy(o_wT, logits)
                else:
                    negmax = small.tile([128, 1], f32, tag="negmax")
                    nc.vector.tensor_reduce(
                        negmax, logits, axis=mybir.AxisListType.X,
                        op=ALU.max, negate=True,
                    )
                    lsh = wtmp.tile([C, V], f32, tag="lsh", name="lsh")
                    nc.vector.tensor_scalar_add(lsh, logits, scalar1=negmax)
                    wT = wtmp.tile([C, V], bf16, tag="wT")
                    sumexp = small.tile([128, 1], f32, tag="sumexp")
                    nc.scalar.activation(
                        wT, lsh, AF.Exp, bias=0.0, scale=1.0,
                        accum_out=sumexp,
                    )
                    rinv = small.tile([128, 1], f32, tag="rinv")
                    nc.vector.reciprocal(rinv, sumexp)
                    nc.vector.tensor_scalar_mul(wT, wT, scalar1=rinv)
                    w_nat[w] = singles.tile([128, V // 128, C], bf16,
                                            tag=f"{w}w", name=f"{w}w_nat")
                    nc.sync.dma_start(w_nat[w], wT, transpose=True)

            # ---- recurrence state (ping-pong)
            S = [
                singles.tile([C, C], bf16, tag="sA", name="sA"),
                singles.tile([C, C], bf16, tag="sB", name="sB"),
            ]
            nc.vector.memset(S[0], 0.0)

            for blk in range(NBLK):
                t0 = blk * BLK
                # block of x^T: one SWDGE casting DMA f32->bf16.
                # xt[p, vb, t] = x[t0+t, 128*vb+p]
                xt = xt_pool.tile([128, V // 128, BLK], bf16, tag="xt")
                nc.gpsimd.dma_start(xt, xT_t[:, :, t0:t0 + BLK])

                # projections, column-major: (C, BLK)
                psq = pproj.tile([C, BLK], f32, tag="pproj")
                for i in range(V // 128):
                    nc.tensor.matmul(
                        psq, w_nat["q"][:, i, :], xt[:, i, :],
                        start=(i == 0), stop=(i == V // 128 - 1),
                    )
                qb = blk_pool.tile([C, BLK], bf16, tag="qb")
                nc.vector.tensor_mul(qb, psq, dq)  # fold d^j into Q

                psk = pproj.tile([C, BLK], f32, tag="pproj")
                for i in range(V // 128):
                    nc.tensor.matmul(
                        psk, w_nat["k"][:, i, :], xt[:, i, :],
                        start=(i == 0), stop=(i == V // 128 - 1),
                    )
                kb = blk_pool.tile([C, BLK], bf16, tag="kb")
                nc.scalar.copy(kb, psk)

                psv = pproj.tile([C, BLK], f32, tag="pproj")
                for i in range(V // 128):
                    nc.tensor.matmul(
                        psv, w_nat["v"][:, i, :], xt[:, i, :],
                        start=(i == 0), stop=(i == V // 128 - 1),
                    )
                vb = blk_pool.tile([C, BLK], bf16, tag="vb")
                nc.scalar.copy(vb, psv)

                # row-major K/V for the contractions over s (xbar transpose,
                # spread across the two HWDGE queues)
                krm = rm_pool.tile([128, NCH, C], bf16, tag="krm")
                nc.sync.dma_start(krm, kb, transpose=True)
                vrm = rm_pool.tile([128, NCH, C], bf16, tag="vrm")
                nc.scalar.dma_start(vrm, vb, transpose=True)

                for ci in range(NCH):
                    ch = blk * NCH + ci
                    cs = slice(ci * L, (ci + 1) * L)
                    s_cur = S[ch % 2]
                    s_nxt = S[(ch + 1) % 2]

                    # scores (s, j) = K^T Q~, then decay+causal mask
                    psa = pa_pool.tile([L, L], f32, tag="pa")
                    nc.tensor.matmul(psa, kb[:, cs], qb[:, cs],
                                     start=True, stop=True)
                    am = ch_pool.tile([L, L], bf16, tag="am")
                    nc.vector.tensor_mul(am, psa, dm)

                    # r (d, j) = inter-chunk (state) + intra-chunk part
                    psr = pr_pool.tile([C, L], f32, tag="pr")
                    nc.tensor.matmul(psr, s_cur, qb[:, cs],
                                     start=True, stop=False)
                    nc.tensor.matmul(psr, vrm[:, ci, :], am,
                                     start=False, stop=True)
                    rcm = ch_pool.tile([C, L], bf16, tag="rcm")
                    nc.scalar.copy(rcm, psr)

                    # state update: S' = d^L S + (K*d^(L-1-s))^T V
                    khat = ch_pool.tile([L, C], bf16, tag="khat")
                    nc.vector.tensor_scalar_mul(khat, krm[:, ci, :], scalar1=dlk)
                    pss = ps_pool.tile([C, C], f32, tag="ps")
                    nc.tensor.matmul(pss, khat, vrm[:, ci, :],
                                     start=True, stop=True)
                    nc.vector.scalar_tensor_tensor(
                        s_nxt, s_cur, dl, pss, op0=ALU.mult, op1=ALU.add,
                    )

                    # output projection: out(j, v) = r^T @ o_w^T
                    pso = pwlog.tile([L, V], f32, tag="wlog", name="pso")
                    out_sb = out_pool.tile([L, V], f32, tag="outsb")
                    for h in range(V // 512):
                        nc.tensor.matmul(
                            pso[:, h * 512:(h + 1) * 512],
                            rcm, o_wT[:, h * 512:(h + 1) * 512],
                            start=True, stop=True,
                        )
                        if h == 0:
                            nc.vector.tensor_copy(
                                out_sb[:, h * 512:(h + 1) * 512],
                                pso[:, h * 512:(h + 1) * 512])
                        else:
                            nc.scalar.copy(
                                out_sb[:, h * 512:(h + 1) * 512],
                                pso[:, h * 512:(h + 1) * 512])
                    nc.gpsimd.dma_start(
                        out_d[ch * L:(ch + 1) * L, :], out_sb,
                    )

    if not nc.is_finalized():
        nc.finalize()
    return nc


def _tables(decay, out_scale):
    d = float(1.0 / (1.0 + np.exp(-np.float64(decay))))
    j = np.arange(L, dtype=np.float64)
    dq = np.broadcast_to(
        np.tile(d ** j, BLK // L)[None, :], (128, BLK)
    ).astype(np.float32)
    s = np.arange(L, dtype=np.float64)
    dm = np.where(
        s[:, None] < j[None, :], d ** (-s[:, None] - 1.0), 0.0
    ).astype(np.float32)
    dlk = (d ** (L - 1.0 - s))[:, None].astype(np.float32)
    dl = np.full((128, 1), d ** L, dtype=np.float32)
    return np.ascontiguousarray(dq), np.ascontiguousarray(dm), \
        np.ascontiguousarray(dlk), dl


def kernel(x, basis, q_coeffs, k_coeffs, v_coeffs, o_coeffs, decay, out_scale):
    from concourse.bass_utils import run_bass_kernel_spmd
    import os

    if "nc" not in _cache:
        _cache["nc"] = _build()
    nc = _cache["nc"]

    bf = ml_dtypes.bfloat16
    x = np.asarray(x, dtype=np.float32)
    xT = np.ascontiguousarray(np.swapaxes(x, 1, 2))  # (B, V, T)
    basisT = np.ascontiguousarray(np.asarray(basis, np.float32).T).astype(bf)
    qcT = np.ascontiguousarray(np.asarray(q_coeffs, np.float32).T).astype(bf)
    kcT = np.ascontiguousarray(np.asarray(k_coeffs, np.float32).T).astype(bf)
    vcT = np.ascontiguousarray(np.asarray(v_coeffs, np.float32).T).astype(bf)
    ocT = np.ascontiguousarray(
        (np.asarray(o_coeffs, np.float32) * np.float32(out_scale)).T
    ).astype(bf)
    dq, dm, dlk, dl = _tables(decay, out_scale)

    shared = {
        "basisT": basisT, "qcT": qcT, "kcT": kcT, "vcT": vcT, "ocT": ocT,
        "dq": dq, "dm": dm, "dlk": dlk, "dl": dl,
    }
    in_maps = [dict(shared, xT=xT[b]) for b in range(B)]

    trace = bool(int(os.environ.get("KERNEL_TRACE", "0")))
    res = run_bass_kernel_spmd(nc, in_maps, core_ids=list(range(B)), trace=trace)
    _cache["exec_time_ns"] = res.exec_time_ns
    return np.stack([res.results[b]["out"] for b in range(B)], axis=0)


# revision 12
# speedup vs baseline: 1.1015x; 1.1015x over previous
"""Trainium2 Bass kernel for AssociativeMemoryStep (decayed linear attention).

Math (per batch b):
    q_w = softmax(basis @ q_coeffs.T, axis=0)          # (V, C)
    k_w, v_w likewise; o_w = basis @ o_coeffs.T        # (V, C)
    q/k/v = x @ {q,k,v}_w                              # (T, C)
    M_t = d*M_{t-1} + k_t v_t^T ;  r_t = q_t^T M_{t-1} # read-before-write
    out = (r @ o_w.T) * out_scale                      # (T, V)

Strategy: data-parallel over B (8 batches -> 8 NeuronCores). Inside each
core the T=2048 sequence is processed in 16 chunks of L=128 with the
standard chunked linear-attention decomposition:
    r_chunk = (Q*d^j) @ S  +  ((K^T (Q*d^j)) .* DM) applied to V
    S_new   = d^L * S + (K*d^(L-1-s))^T V
where DM[s,j] = d^(-s-1) * [s<j] folds the decay+causal mask into one
elementwise multiply of the (s,j) score matrix.

Layout: x is passed host-transposed as xT (V, T) f32 so the V-contraction
of the projections needs no on-device transpose; a SWDGE casting DMA
loads each 512-column block straight into bf16 SBUF in (p, vb, t) form.
Q/K are projected column-major (C on partitions, T free, N=512); V is
projected row-major per chunk (lhsT = xT slice) since only the row-major
form is consumed. Row-major K comes from one xbar transpose per block.

Engine discipline (keeps every instruction at <=1 cross-engine wait so
Bacc's EVENT_SEMAPHORE splitting stays off the critical path): TensorE
does all matmuls; DVE does every PSUM->SBUF conversion whose output
feeds TensorE; ACT does the conversions whose output feeds the store
DMAs; SWDGE (gpsimd) does all plain DMAs; the two HWDGE queues carry
only xbar transposes.
"""

import numpy as np
import ml_dtypes

B, T, V, C, NB2 = 8, 2048, 1024, 128, 64
L = 128               # recurrence chunk
BLK = 512             # projection block (free dim of proj matmuls)
NBLK = T // BLK       # 4
NCH = BLK // L        # 4 chunks per block

_cache = {}


def _build():
    import concourse.bass as bass
    import concourse.tile as tile
    from concourse import bacc, mybir

    f32 = mybir.dt.float32
    bf16 = mybir.dt.bfloat16
    AF = mybir.ActivationFunctionType
    ALU = mybir.AluOpType

    nc = bacc.Bacc()

    xT_d = nc.declare_dram_parameter("xT", [V, T], f32, isOutput=False)
    basisT_d = nc.declare_dram_parameter("basisT", [NB2, V], bf16, isOutput=False)
    coeff_d = {
        w: nc.declare_dram_parameter(f"{w}cT", [NB2, C], bf16, isOutput=False)
        for w in ("q", "k", "v", "o")
    }
    dq_d = nc.declare_dram_parameter("dq", [128, BLK], f32, isOutput=False)
    dm_d = nc.declare_dram_parameter("dm", [128, 128], f32, isOutput=False)
    dlk_d = nc.declare_dram_parameter("dlk", [128, 1], f32, isOutput=False)
    dl_d = nc.declare_dram_parameter("dl", [128, 1], f32, isOutput=False)
    out_d = nc.declare_dram_parameter("out", [T, V], f32, isOutput=True)

    xT_t = xT_d[:].rearrange("(vb p) t -> p vb t", p=128)  # (128, 8, T)

    with tile.TileContext(nc) as tc:
        with (
            tc.tile_pool(name="singles", bufs=1) as singles,
            tc.tile_pool(name="wtmp", bufs=3) as wtmp,
            tc.tile_pool(name="small", bufs=4) as small,
            tc.tile_pool(name="xt", bufs=3) as xt_pool,
            tc.tile_pool(name="blk", bufs=3) as blk_pool,
            tc.tile_pool(name="rm", bufs=4) as rm_pool,
            tc.tile_pool(name="ch", bufs=3) as ch_pool,
            tc.tile_pool(name="outsb", bufs=3) as out_pool,
            tc.tile_pool(name="pwlog", bufs=1, space="PSUM") as pwlog,
            tc.tile_pool(name="pproj", bufs=2, space="PSUM") as pproj,
            tc.tile_pool(name="pa", bufs=1, space="PSUM") as pa_pool,
            tc.tile_pool(name="pr", bufs=1, space="PSUM") as pr_pool,
            tc.tile_pool(name="ps", bufs=1, space="PSUM") as ps_pool,
            tc.tile_pool(name="pv", bufs=1, space="PSUM") as pv_pool,
        ):
            # ---- small params -> SBUF
            basisT = singles.tile([NB2, V], bf16, tag="basisT")
            nc.gpsimd.dma_start(basisT, basisT_d[:])
            coeff = {}
            for w in ("q", "k", "v", "o"):
                coeff[w] = singles.tile([NB2, C], bf16, tag=f"{w}cT", name=f"{w}cT_sb")
                nc.gpsimd.dma_start(coeff[w], coeff_d[w][:])
            dq = singles.tile([128, BLK], f32, tag="dq")
            nc.gpsimd.dma_start(dq, dq_d[:])
            dm = singles.tile([128, 128], f32, tag="dm")
            nc.gpsimd.dma_start(dm, dm_d[:])
            dlk = singles.tile([128, 1], f32, tag="dlk")
            nc.gpsimd.dma_start(dlk, dlk_d[:])
            dl = singles.tile([128, 1], f32, tag="dl")
            nc.gpsimd.dma_start(dl, dl_d[:])

            # ---- Fourier-parameterized projection weights, on device.
            # logits^T (C,V) = coeffs @ basis.T ; softmax along free (V).
            # Logits are O(1) (coeffs ~0.02), so exp needs no max-shift.
            w_nat = {}   # (128, 8, 128) bf16: w_nat[p, i, c] = w[(128i+p), c]
            o_wT = None  # (C, V) bf16
            for w in ("q", "k", "v", "o"):
                logits = pwlog.tile([128, V], f32, tag="wlog")
                for h in range(V // 512):
                    nc.tensor.matmul(
                        logits[:, h * 512:(h + 1) * 512],
                        coeff[w],
                        basisT[:, h * 512:(h + 1) * 512],
                        start=True, stop=True,
                    )
                if w == "o":
                    o_wT = singles.tile([C, V], bf16, tag="o_wT")
                    nc.scalar.copy(o_wT, logits)
                else:
                    wT = wtmp.tile([C, V], bf16, tag="wT")
                    sumexp = small.tile([128, 1], f32, tag="sumexp")
                    nc.scalar.activation(
                        wT, logits, AF.Exp, bias=0.0, scale=1.0,
                        accum_out=sumexp,
                    )
                    rinv = small.tile([128, 1], f32, tag="rinv")
                    nc.vector.reciprocal(rinv, sumexp)
                    nc.vector.tensor_scalar_mul(wT, wT, scalar1=rinv)
                    w_nat[w] = singles.tile([128, V // 128, C], bf16,
                                            tag=f"{w}w", name=f"{w}w_nat")
                    nc.sync.dma_start(w_nat[w], wT, transpose=True)

            # ---- recurrence state (ping-pong)
            S = [
                singles.tile([C, C], bf16, tag="sA", name="sA"),
                singles.tile([C, C], bf16, tag="sB", name="sB"),
            ]
            nc.vector.memset(S[0], 0.0)

            for blk in range(NBLK):
                t0 = blk * BLK
                # block of x^T: one SWDGE casting DMA f32->bf16.
                # xt[p, vb, t] = x[t0+t, 128*vb+p]
                xt = xt_pool.tile([128, V // 128, BLK], bf16, tag="xt")
                nc.gpsimd.dma_start(xt, xT_t[:, :, t0:t0 + BLK])

                # Q/K projections, column-major: (C, BLK)
                psq = pproj.tile([C, BLK], f32, tag="pproj")
                for i in range(V // 128):
                    nc.tensor.matmul(
                        psq, w_nat["q"][:, i, :], xt[:, i, :],
                        start=(i == 0), stop=(i == V // 128 - 1),
                    )
                qb = blk_pool.tile([C, BLK], bf16, tag="qb")
                nc.vector.tensor_mul(qb, psq, dq)  # fold d^j into Q

                psk = pproj.tile([C, BLK], f32, tag="pproj")
                for i in range(V // 128):
                    nc.tensor.matmul(
                        psk, w_nat["k"][:, i, :], xt[:, i, :],
                        start=(i == 0), stop=(i == V // 128 - 1),
                    )
                kb = blk_pool.tile([C, BLK], bf16, tag="kb")
                nc.vector.tensor_copy(kb, psk)

                # row-major K via xbar transpose; khat for the whole block
                krm = rm_pool.tile([128, NCH, C], bf16, tag="krm")
                nc.sync.dma_start(krm, kb, transpose=True)
                khat = rm_pool.tile([128, NCH, C], bf16, tag="khat")
                nc.vector.tensor_scalar_mul(khat, krm, scalar1=dlk)

                for ci in range(NCH):
                    ch = blk * NCH + ci
                    cs = slice(ci * L, (ci + 1) * L)
                    s_cur = S[ch % 2]
                    s_nxt = S[(ch + 1) % 2]

                    # V row-major, projected directly: (t, c)
                    psv = pv_pool.tile([L, C], f32, tag="pv")
                    for i in range(V // 128):
                        nc.tensor.matmul(
                            psv, xt[:, i, cs], w_nat["v"][:, i, :],
                            start=(i == 0), stop=(i == V // 128 - 1),
                        )
                    vrm = ch_pool.tile([L, C], bf16, tag="vrm")
                    nc.vector.tensor_copy(vrm, psv)

                    # scores (s, j) = K^T Q~, then decay+causal mask
                    psa = pa_pool.tile([L, L], f32, tag="pa")
                    nc.tensor.matmul(psa, kb[:, cs], qb[:, cs],
                                     start=True, stop=True)
                    am = ch_pool.tile([L, L], bf16, tag="am")
                    nc.vector.tensor_mul(am, psa, dm)

                    # r (d, j) = intra-chunk part + inter-chunk (state) part
                    psr = pr_pool.tile([C, L], f32, tag="pr")
                    nc.tensor.matmul(psr, vrm, am,
                                     start=True, stop=False)
                    nc.tensor.matmul(psr, s_cur, qb[:, cs],
                                     start=False, stop=True)
                    rcm = ch_pool.tile([C, L], bf16, tag="rcm")
                    nc.vector.tensor_copy(rcm, psr)

                    # state update: S' = d^L S + (K*d^(L-1-s))^T V
                    pss = ps_pool.tile([C, C], f32, tag="ps")
                    nc.tensor.matmul(pss, khat[:, ci, :], vrm,
                                     start=True, stop=True)
                    nc.vector.scalar_tensor_tensor(
                        s_nxt, s_cur, dl, pss, op0=ALU.mult, op1=ALU.add,
                    )

                    # output projection: out(j, v) = r^T @ o_w^T
                    pso = pwlog.tile([L, V], f32, tag="wlog", name="pso")
                    out_sb = out_pool.tile([L, V], f32, tag="outsb")
                    for h in range(V // 512):
                        nc.tensor.matmul(
                            pso[:, h * 512:(h + 1) * 512],
                            rcm, o_wT[:, h * 512:(h + 1) * 512],
                            start=True, stop=True,
                        )
                        nc.scalar.copy(
                            out_sb[:, h * 512:(h + 1) * 512],
                            pso[:, h * 512:(h + 1) * 512])
                    nc.gpsimd.dma_start(
                        out_d[ch * L:(ch + 1) * L, :], out_sb,
                    )

    if not nc.is_finalized():
        nc.finalize()
    return nc


def _tables(decay, out_scale):
    d = float(1.0 / (1.0 + np.exp(-np.float64(decay))))
    j = np.arange(L, dtype=np.float64)
    dq = np.broadcast_to(
        np.tile(d ** j, BLK // L)[None, :], (128, BLK)
    ).astype(np.float32)
    s = np.arange(L, dtype=np.float64)
    dm = np.where(
        s[:, None] < j[None, :], d ** (-s[:, None] - 1.0), 0.0
    ).astype(np.float32)
    dlk = (d ** (L - 1.0 - s))[:, None].astype(np.float32)
    dl = np.full((128, 1), d ** L, dtype=np.float32)
    return np.ascontiguousarray(dq), np.ascontiguousarray(dm), \
        np.ascontiguousarray(dlk), dl


def kernel(x, basis, q_coeffs, k_coeffs, v_coeffs, o_coeffs, decay, out_scale):
    from concourse.bass_utils import run_bass_kernel_spmd
    import os

    if "nc" not in _cache:
        _cache["nc"] = _build()
    nc = _cache["nc"]

    bf = ml_dtypes.bfloat16
    x = np.asarray(x, dtype=np.float32)
    xT = np.ascontiguousarray(np.swapaxes(x, 1, 2))  # (B, V, T)
    basisT = np.ascontiguousarray(np.asarray(basis, np.float32).T).astype(bf)
    qcT = np.ascontiguousarray(np.asarray(q_coeffs, np.float32).T).astype(bf)
    kcT = np.ascontiguousarray(np.asarray(k_coeffs, np.float32).T).astype(bf)
    vcT = np.ascontiguousarray(np.asarray(v_coeffs, np.float32).T).astype(bf)
    ocT = np.ascontiguousarray(
        (np.asarray(o_coeffs, np.float32) * np.float32(out_scale)).T
    ).astype(bf)
    dq, dm, dlk, dl = _tables(decay, out_scale)

    shared = {
        "basisT": basisT, "qcT": qcT, "kcT": kcT, "vcT": vcT, "ocT": ocT,
        "dq": dq, "dm": dm, "dlk": dlk, "dl": dl,
    }
    in_maps = [dict(shared, xT=xT[b]) for b in range(B)]

    trace = bool(int(os.environ.get("KERNEL_TRACE", "0")))
    res = run_bass_kernel_spmd(nc, in_maps, core_ids=list(range(B)), trace=trace)
    _cache["exec_time_ns"] = res.exec_time_ns
    return np.stack([res.results[b]["out"] for b in range(B)], axis=0)
